# revision 10
# baseline (speedup 1.0000x reference)
"""3-layer GCN + gene-pair MLP on 8 Trainium2 NeuronCores (Bass/Tile).

Strategy (v2)
-------------
Nodes are sharded contiguously across the 8 cores by dst (12500 nodes each).
Edges live on the core that owns their dst node, bucketed by (dst-tile, band)
and padded only to the max count over cores.  Layer tables are node-major
bf16 rows padded to 256 B; layer-1's table (x*out_isqrt)@W1 is host-folded
(a linear transform of the inputs), so the kernel starts gathering
immediately.  Each layer:
  1. each core gathers table[src] rows for its edges with the SWDGE
     dma_gather, one gather per (tile, band) so the per-(t,b) padding can be
     trimmed via num_idxs_reg + trailing negative indices,
  2. aggregation is a one-hot matmul: S[e, v] = (dst_local[e] == v) built by
     a single DVE op (pad slots carry dl = -1 so their S row is zero), then
     aggT[f, v] += G[e, f]^T @ S accumulated in PSUM per 128-node tile,
  3. both GCN normalizations are folded into per-node scalings:
     out_isqrt premultiplies the staged table (host-side for layer 1,
     fused into the post-agg scale for layers 2/3), in_isqrt postmultiplies
     the aggregate; with zero biases relu(agg*isq)*osq == relu(agg*isq*osq)
     so one wide DVE multiply + relu per layer suffices.
  4. hw = h @ W is staged node-major and AllGathered into a shared full
     table for the next layer's gathers.  The first few gathers of the next
     layer are issued as SWDGE prepare_only on spare queues during the
     AllGather so the Q7 descriptor generator never idles; their triggers
     carry the deferred table dependency.
After layer 3 the kernel stages u = h3 @ Wfc1[:64], v = h3 @ Wfc1[64:] as one
packed [u|v] table; pairs gather u[gene1], v[gene2], and the 2-class softmax
collapses to sigmoid(z @ (Wfc2[:,1]-Wfc2[:,0]) + db).
"""
import sys
import os

sys.path.insert(0, "/opt/trn_rl_repo")

import numpy as np
import ml_dtypes

import concourse.bacc as bacc
import concourse.mybir as mybir
import concourse.tile as tile
from concourse.bass_utils import run_bass_kernel_spmd

bf16 = mybir.dt.bfloat16
f32 = mybir.dt.float32

R = int(os.environ.get("GCN_R", "8"))  # cores
V = 128          # nodes per aggregation tile
GT = int(os.environ.get("GCN_GT", "14"))  # tiles per gather group
MAXBAND = 30000  # int16-addressable rows per gather band (< 32768)
NPRE = int(os.environ.get("GCN_NPRE", "18"))   # prep-ahead gathers per layer
NPREP = int(os.environ.get("GCN_NPREP", "3"))  # prep-ahead pair buckets
GB = int(os.environ.get("GCN_GB", "4"))        # G/S pool depth

_BF = ml_dtypes.bfloat16


def _ceil(a, b):
    return -(-a // b)


def _wrap_idx(flat):
    """dma_gather index layout: position j -> [j % 16, j // 16], x8 partitions."""
    n = len(flat)
    assert n % 128 == 0
    arr = np.ascontiguousarray(flat.reshape(n // 16, 16).T.astype(np.int16))
    return np.tile(arr, (8, 1))


class _Plan:
    pass


def _make_plan(n_nodes, src, dst, gene1, gene2):
    p = _Plan()
    N = n_nodes
    NP = gene1.shape[0]
    p.N, p.NP = N, NP
    p.NPR = _ceil(N, R)               # nodes per rank
    p.TPR = _ceil(p.NPR, 128)         # node tiles per rank
    p.ROWS_PR = p.TPR * 128           # table rows per rank
    p.TOT_ROWS = p.ROWS_PR * R
    p.NB = max(1, _ceil(p.TOT_ROWS, MAXBAND))
    p.BSZ = _ceil(p.TOT_ROWS, p.NB)   # rows per band (last may be short)
    assert p.BSZ < 32768
    p.PPR = _ceil(NP, R)              # pairs per rank

    def row_of(n):
        r = n // p.NPR
        l = n - r * p.NPR
        return p.ROWS_PR * r + p.TPR * (l % 128) + (l // 128)

    p.row_of = row_of

    # ---- edge structure (shared across the 3 layers) ----
    own = (dst // p.NPR).astype(np.int64)
    loc = dst - own * p.NPR
    tl = loc // 128                      # tile within rank
    dl = (loc % 128).astype(np.float32)  # one-hot column
    rs = row_of(src)
    band = rs // p.BSZ
    ridx = (rs - band * p.BSZ).astype(np.int64)

    NBt = p.NB
    bid = (own * p.TPR + tl) * NBt + band
    counts = np.bincount(bid, minlength=R * p.TPR * NBt).reshape(R, p.TPR, NBt)
    p.cnts = counts
    p.Lmax = counts.max(axis=0)                    # [TPR, NB] true max count
    p.Pch = _ceil(p.Lmax, 128)                     # chunks per (tile, band)
    p.Pch = np.maximum(p.Pch, 1)

    # column/run offsets in (group, band, tile) order; GT tiles per gather
    p.NG = _ceil(p.TPR, GT)
    p.col_run = np.zeros((p.TPR, NBt), np.int64)
    p.gathers = []                                 # (g, b, c0, nch)
    col = 0
    for g in range(p.NG):
        ts = range(g * GT, min((g + 1) * GT, p.TPR))
        for b in range(NBt):
            c0 = col
            for t in ts:
                p.col_run[t, b] = col
                col += p.Pch[t, b]
            p.gathers.append((g, b, c0, col - c0))
    p.CT = int(col)
    E_pad = p.CT * 128

    # per-core flat slots
    order = np.argsort(bid, kind="stable")
    bid_s = bid[order]
    own_s = own[order]
    uniq, first = np.unique(bid_s, return_index=True)
    start_map = np.zeros(R * p.TPR * NBt, np.int64)
    start_map[uniq] = first
    i_within = np.arange(len(order)) - start_map[bid_s]
    tl_s, band_s = tl[order], band[order]
    slot = p.col_run[tl_s, band_s] * 128 + i_within

    p.idx2 = np.zeros((R, 128, p.CT * 8), np.int16)
    p.dl2 = np.zeros((R, 128, p.CT), _BF)
    ridx_s, dl_ss = ridx[order], dl[order]

    # pad slots fetch row 0 of their band (valid) with dl = -1 so their
    # one-hot row is zero and they contribute nothing.
    for r in range(R):
        m = own_s == r
        idx_flat = np.zeros(E_pad, np.int64)
        dl_flat = np.full(E_pad, -1.0, np.float32)
        idx_flat[slot[m]] = ridx_s[m]
        dl_flat[slot[m]] = dl_ss[m]
        p.dl2[r] = dl_flat.reshape(p.CT, 128).T.astype(_BF)
        blocks = []
        for (_, _, c0, nch) in p.gathers:
            blocks.append(_wrap_idx(idx_flat[c0 * 128:(c0 + nch) * 128]))
        p.idx2[r] = np.hstack(blocks)

    # ---- pair structure ----
    g1r, g2r = row_of(gene1), row_of(gene2)
    pb = (g1r // p.BSZ) * NBt + (g2r // p.BSZ)
    pown = np.arange(NP) // p.PPR
    NBK = NBt * NBt
    pcnt = np.bincount(pown * NBK + pb, minlength=R * NBK).reshape(R, NBK)
    p.Lmaxp = pcnt.max(axis=0)
    p.Pchp = np.maximum(_ceil(p.Lmaxp, 128), 1)    # chunks per bucket
    p.pcol = np.concatenate([[0], np.cumsum(p.Pchp)])
    p.PCT = int(p.pcol[-1])
    PP_pad = p.PCT * 128

    pbid = pown * NBK + pb
    porder = np.argsort(pbid, kind="stable")
    pbid_s = pbid[porder]
    pown_s = pown[porder]
    uq, fs = np.unique(pbid_s, return_index=True)
    smap = np.zeros(R * NBK, np.int64)
    smap[uq] = fs
    pi_within = np.arange(NP) - smap[pbid_s]
    pslot = p.pcol[pb[porder]] * 128 + pi_within

    pbase = np.zeros(PP_pad, np.int64)
    for bkt in range(NBK):
        c0, lm = int(p.pcol[bkt]), int(p.Lmaxp[bkt])
        pbase[c0 * 128 + lm: (c0 + p.Pchp[bkt]) * 128] = -1

    p.pidx1 = np.zeros((R, 128, p.PCT * 8), np.int16)
    p.pidx2 = np.zeros((R, 128, p.PCT * 8), np.int16)
    p.perm = np.full((R, PP_pad), -1, np.int64)
    r1 = (g1r - (g1r // p.BSZ) * p.BSZ)[porder]
    r2 = (g2r - (g2r // p.BSZ) * p.BSZ)[porder]
    for r in range(R):
        m = pown_s == r
        f1 = pbase.copy()
        f2 = pbase.copy()
        f1[pslot[m]] = r1[m]
        f2[pslot[m]] = r2[m]
        p.perm[r][pslot[m]] = porder[m]
        b1s, b2s = [], []
        for bkt in range(NBK):
            c0, nch = int(p.pcol[bkt]), int(p.Pchp[bkt])
            b1s.append(_wrap_idx(f1[c0 * 128:(c0 + nch) * 128]))
            b2s.append(_wrap_idx(f2[c0 * 128:(c0 + nch) * 128]))
        p.pidx1[r] = np.hstack(b1s)
        p.pidx2[r] = np.hstack(b2s)
    return p


def _build(p, any_bz):
    """Build the SPMD Bass program for plan `p`."""
    STOP = int(os.environ.get("GCN_STOP", "9"))
    PREP = bool(os.environ.get("GCN_PREP"))
    nc = bacc.Bacc("TRN2", num_devices=R, num_swdge_queues=4)
    NBt, NBK = p.NB, p.NB * p.NB

    tbl1_d = nc.dram_tensor("tbl1", [p.TOT_ROWS, 128], bf16, kind="ExternalInput")
    idx_d = nc.dram_tensor("idxE", [128, p.CT * 8], mybir.dt.int16, kind="ExternalInput")
    dl_d = nc.dram_tensor("dlE", [128, p.CT], bf16, kind="ExternalInput")
    pi1_d = nc.dram_tensor("pidx1", [128, p.PCT * 8], mybir.dt.int16, kind="ExternalInput")
    pi2_d = nc.dram_tensor("pidx2", [128, p.PCT * 8], mybir.dt.int16, kind="ExternalInput")
    Ws_d = nc.dram_tensor("Ws", [64, 4, 64], bf16, kind="ExternalInput")
    sc12_d = nc.dram_tensor("sc12", [128, p.TPR], f32, kind="ExternalInput")
    sc3_d = nc.dram_tensor("sc3", [128, p.TPR], f32, kind="ExternalInput")
    wdbd_d = nc.dram_tensor("wdbd", [128, 65], f32, kind="ExternalInput")
    iota_d = nc.dram_tensor("iotain", [128, V], bf16, kind="ExternalInput")
    bz_d = nc.dram_tensor("bz", [128, 64], f32, kind="ExternalInput") if any_bz else None
    pout_d = nc.dram_tensor("pout", [128, p.PCT, 2], f32, kind="ExternalOutput")

    rg = [list(range(R))]
    # (tile, band) gather order
    gorder = [(t, b) for t in range(p.TPR) for b in range(NBt)]

    with tile.TileContext(nc) as tc:
        with tc.tile_pool(name="dloc", bufs=1, space="DRAM") as dloc, \
             tc.tile_pool(name="sb", bufs=1) as sb, \
             tc.tile_pool(name="ps", bufs=1, space="PSUM") as ps:

            stage_dram = dloc.tile([128, p.TPR, 128], bf16)
            fulls = [dloc.tile([p.TOT_ROWS, 128], bf16, tag=f"full{i}",
                               name=f"full{i}", addr_space="Shared")
                     for i in range(3)]

            iota_t = sb.tile([128, V], bf16)
            idx_t = sb.tile([128, p.CT * 8], mybir.dt.int16)
            dl_t = sb.tile([128, p.CT], bf16)
            pi1_t = sb.tile([128, p.PCT * 8], mybir.dt.int16)
            pi2_t = sb.tile([128, p.PCT * 8], mybir.dt.int16)
            Ws_t = sb.tile([64, 4, 64], bf16)
            sc12_t = sb.tile([128, p.TPR], f32)
            sc3_t = sb.tile([128, p.TPR], f32)
            wdbd_t = sb.tile([128, 65], f32)
            for t_, d_ in ((iota_t, iota_d), (idx_t, idx_d), (dl_t, dl_d),
                           (pi1_t, pi1_d), (pi2_t, pi2_d),
                           (Ws_t, Ws_d), (sc12_t, sc12_d), (sc3_t, sc3_d),
                           (wdbd_t, wdbd_d)):
                nc.sync.dma_start(out=t_[:], in_=d_[:])
            bz_t = None
            if any_bz:
                bz_t = sb.tile([128, 64], f32)
                nc.sync.dma_start(out=bz_t[:], in_=bz_d[:])

            stage_sb = sb.tile([128, p.TPR, 128], bf16)
            nc.vector.memset(stage_sb[:], 0.0)

            maxPch = max(nch for (_, _, _, nch) in p.gathers)
            import itertools
            _gseq = itertools.count()

            def g_tile():
                return sb.tile([128, maxPch, 128], bf16, tag="G", bufs=GB,
                               name=f"G{next(_gseq)}")

            def s_tile(nch):
                return sb.tile([128, nch, V], bf16, tag="S", bufs=4,
                               name=f"S{next(_gseq)}")

            def emit_gather(table, gi, prepare=False, sem=None, queue=0):
                (_, b, c0, nch) = p.gathers[gi]
                Gt = g_tile()
                lo = b * p.BSZ
                hi = min(lo + p.BSZ, p.TOT_ROWS)
                nc.gpsimd.dma_gather(
                    out_ap=Gt[:, 0:nch, :], in_ap=table[lo:hi, :],
                    idxs_ap=idx_t[:, c0 * 8:(c0 + nch) * 8],
                    num_idxs=nch * 128, num_idxs_reg=nch * 128,
                    elem_size=128, single_packet=False,
                    prepare_only=prepare, sem=sem, queue_num=queue)
                return Gt

            def wmm_stage(src_tile, wi, half, sc_t):
                """hw[:, t] = sc * src_tile[:, t*128:...].T @ Ws[wi] into stage.

                sc is the per-node (per-partition here) norm fold: relu(s*x)
                = s*relu(x) and row scaling commutes with @W, so both GCN
                norms land here as one fused tensor_scalar multiply."""
                for t in range(p.TPR):
                    pm = ps.tile([128, 64], f32, tag="wm", space="PSUM", bufs=2)
                    nc.tensor.matmul(out=pm[:], lhsT=src_tile[:, t * 128:(t + 1) * 128],
                                     rhs=Ws_t[:, wi, :], start=True, stop=True)
                    nc.vector.tensor_scalar(
                        out=stage_sb[:, t, half * 64:half * 64 + 64], in0=pm[:],
                        scalar1=sc_t[:, t:t + 1], scalar2=None,
                        op0=mybir.AluOpType.mult)

            pre_G = {}

            def emit_preps(l, table):
                if not PREP or NPRE <= 0:
                    return {}
                per_q = _ceil(NPRE, 3)
                used = {}
                for i in range(min(NPRE, len(p.gathers))):
                    q = 1 + i // per_q
                    sem = nc.alloc_semaphore(f"prep_l{l}_{i}")
                    Gt = emit_gather(table, i, prepare=True, sem=sem, queue=q)
                    pre_G[(l, i)] = Gt
                    nch = p.gathers[i][3]
                    used.setdefault(q, []).append(Gt[:, 0:nch, :])
                return used

            def emit_layer(l, table):
                hT = sb.tile([64, p.ROWS_PR], bf16, tag="feat", bufs=1,
                             name=f"hT{next(_gseq)}")
                for g in range(p.NG):
                    ts = range(g * GT, min((g + 1) * GT, p.TPR))
                    Gs, Ss, c0s = {}, {}, {}
                    for gi, (gg, b, c0, nch) in enumerate(p.gathers):
                        if gg != g:
                            continue
                        c0s[b] = c0
                        if (l, gi) in pre_G:
                            Gt = pre_G.pop((l, gi))
                        else:
                            Gt = emit_gather(table, gi)
                        St = s_tile(nch)
                        dl_b = dl_t[:, c0:c0 + nch].unsqueeze(2).to_broadcast([128, nch, V])
                        iota_b = iota_t[:].unsqueeze(1).to_broadcast([128, nch, V])
                        nc.vector.tensor_tensor(out=St[:], in0=iota_b, in1=dl_b,
                                                op=mybir.AluOpType.is_equal)
                        Gs[b], Ss[b] = Gt, St
                    for t in ts:
                        acc = ps.tile([64, V], f32, tag="acc", space="PSUM", bufs=2)
                        nch_t = int(p.Pch[t, :].sum())
                        ki = 0
                        for b in range(NBt):
                            base = int(p.col_run[t, b] - c0s[b])
                            for k in range(int(p.Pch[t, b])):
                                nc.tensor.matmul(
                                    out=acc[:],
                                    lhsT=Gs[b][:, base + k, 0:64],
                                    rhs=Ss[b][:, base + k, :],
                                    start=(ki == 0), stop=(ki == nch_t - 1))
                                ki += 1
                        nc.vector.tensor_copy(hT[:, t * 128:(t + 1) * 128], acc[:])
                if l < 2:
                    nc.vector.tensor_scalar_max(hT[:], hT[:], 0.0)
                return hT

            pre_P = {}

            def emit_pair_gather(bkt, which, prepare=False, sem=None, queue=0):
                c0, nch = int(p.pcol[bkt]), int(p.Pchp[bkt])
                lm = int(p.Lmaxp[bkt])
                b = bkt // NBt if which == 0 else bkt % NBt
                pit = pi1_t if which == 0 else pi2_t
                tt = sb.tile([128, nch, 128], bf16, tag="UV", bufs=6,
                             name=f"UV{next(_gseq)}")
                lo = b * p.BSZ
                hi = min(lo + p.BSZ, p.TOT_ROWS)
                nc.gpsimd.dma_gather(
                    out_ap=tt[:], in_ap=fulls[2][lo:hi, :],
                    idxs_ap=pit[:, c0 * 8:(c0 + nch) * 8],
                    num_idxs=nch * 128, num_idxs_reg=lm,
                    elem_size=128, single_packet=False,
                    prepare_only=prepare, sem=sem, queue_num=queue)
                return tt

            # ---- layers ----
            table = tbl1_d
            for l in range(3):
                if STOP < l + 1:
                    break
                hT = emit_layer(l, table)
                if l < 2:
                    wmm_stage(hT, l, 0, sc12_t)
                    nxt = fulls[l]
                else:
                    wmm_stage(hT, 2, 0, sc3_t)   # u = h3 @ Wfc1[:64]
                    wmm_stage(hT, 3, 1, sc3_t)   # v = h3 @ Wfc1[64:]
                    nxt = fulls[2]
                nc.sync.dma_start(out=stage_dram[:], in_=stage_sb[:])
                nc.gpsimd.collective_compute(
                    "AllGather", mybir.AluOpType.bypass, replica_groups=rg,
                    ins=[stage_dram[:]], outs=[nxt[:]])
                # prep-ahead for the next consumer of `nxt`: emitted after the
                # AllGather so the deferred table-read dependency lands on the
                # trigger (the prep itself runs during the collective).
                if l < 2:
                    used_q = emit_preps(l + 1, nxt)
                else:
                    used_q = {}
                    if PREP and NPREP > 0:
                        for bi in range(min(NPREP, NBK)):
                            q = 1 + bi % 3
                            for which in (0, 1):
                                sem = nc.alloc_semaphore(f"prep_p{bi}_{which}")
                                tt = emit_pair_gather(
                                    bi, which, prepare=True, sem=sem, queue=q)
                                pre_P[(bi, which)] = tt
                                used_q.setdefault(q, []).append(tt[:])
                for q in sorted(used_q):
                    nc.gpsimd.trigger_dma(count=None, queue_num=q,
                                          signals_writable=used_q[q])
                table = nxt

            # ---- pair stage ----
            for bkt in range(NBK) if STOP >= 4 else []:
                nch = int(p.Pchp[bkt])
                if (bkt, 0) in pre_P:
                    Ut = pre_P.pop((bkt, 0))
                    Vt = pre_P.pop((bkt, 1))
                else:
                    Ut = emit_pair_gather(bkt, 0)
                    Vt = emit_pair_gather(bkt, 1)
                z = sb.tile([128, nch, 64], f32, tag="z", bufs=2)
                nc.vector.tensor_tensor(out=z[:], in0=Ut[:, :, 0:64],
                                        in1=Vt[:, :, 64:128],
                                        op=mybir.AluOpType.add)
                if any_bz:
                    nc.vector.tensor_tensor(
                        out=z[:], in0=z[:],
                        in1=bz_t[:].unsqueeze(1).to_broadcast([128, nch, 64]),
                        op=mybir.AluOpType.add)
                nc.vector.tensor_scalar_max(z[:], z[:], 0.0)
                nc.vector.tensor_tensor(
                    out=z[:], in0=z[:],
                    in1=wdbd_t[:, 0:64].unsqueeze(1).to_broadcast([128, nch, 64]),
                    op=mybir.AluOpType.mult)
                ds = sb.tile([128, nch], f32, tag="ds", bufs=2)
                nc.vector.tensor_reduce(out=ds[:], in_=z[:],
                                        axis=mybir.AxisListType.X,
                                        op=mybir.AluOpType.add)
                po = sb.tile([128, nch, 2], f32, tag="po", bufs=2)
                nc.scalar.activation(po[:, :, 1:2], ds[:].unsqueeze(2),
                                     mybir.ActivationFunctionType.Sigmoid,
                                     bias=wdbd_t[:, 64:65], scale=1.0)
                nc.vector.tensor_scalar(
                    out=po[:, :, 0:1], in0=po[:, :, 1:2],
                    scalar1=-1.0, scalar2=1.0,
                    op0=mybir.AluOpType.mult, op1=mybir.AluOpType.add)
                c0 = int(p.pcol[bkt])
                nc.sync.dma_start(out=pout_d[:, c0:c0 + nch, :], in_=po[:])
    nc.compile()
    return nc


def _split_excess_waits(nc, max_waits=1):
    """Walrus rejects >1 sem wait on queue instructions; hoist extras onto
    standalone EventSemaphore instructions placed just before."""
    for fn in nc.m.functions:
        for bb in fn.blocks:
            il = bb.instructions
            new_list = []
            changed = False
            for ins in il:
                si = ins.sync_info
                if si is not None and si.on_wait and len(si.on_wait) > max_waits:
                    waits = list(si.on_wait)
                    keep, excess = waits[:max_waits], waits[max_waits:]
                    for gi in range(0, len(excess), max_waits):
                        ev = mybir.InstEventSemaphore(
                            name=f"{ins.name}_wsplit{gi}", ins=[], outs=[])
                        ev.engine = ins.engine
                        ev.sync_info = mybir.SyncInfo(
                            on_wait=excess[gi:gi + max_waits], on_update=[])
                        new_list.append(ev)
                    ins.sync_info = mybir.SyncInfo(
                        on_wait=keep, on_update=list(si.on_update))
                    changed = True
                new_list.append(ins)
            if changed:
                bb.instructions = new_list


def kernel(x, src, dst, gene1, gene2, W1, b1, W2, b2, W3, b3,
           Wfc1, bfc1, Wfc2, bfc2, _trace=False):
    x = np.asarray(x, np.float32)
    src = np.asarray(src, np.int64)
    dst = np.asarray(dst, np.int64)
    gene1 = np.asarray(gene1, np.int64)
    gene2 = np.asarray(gene2, np.int64)
    W1, b1 = np.asarray(W1, np.float32), np.asarray(b1, np.float32)
    W2, b2 = np.asarray(W2, np.float32), np.asarray(b2, np.float32)
    W3, b3 = np.asarray(W3, np.float32), np.asarray(b3, np.float32)
    Wfc1, bfc1 = np.asarray(Wfc1, np.float32), np.asarray(bfc1, np.float32)
    Wfc2, bfc2 = np.asarray(Wfc2, np.float32), np.asarray(bfc2, np.float32)

    assert not (np.any(b1) or np.any(b2) or np.any(b3)), \
        "nonzero GCN biases not supported by the folded-norm fast path"

    N = x.shape[0]
    p = _make_plan(N, src, dst, gene1, gene2)

    # degree norms (host, structural)
    ones = np.ones(len(src), np.float32)
    out_deg = np.clip(np.bincount(src, weights=ones, minlength=N), 1.0, None)
    in_deg = np.clip(np.bincount(dst, weights=ones, minlength=N), 1.0, None)
    osq = (out_deg ** -0.5).astype(np.float32)
    isq = (in_deg ** -0.5).astype(np.float32)

    # layer-1 table host-folded: (x * osq) @ W1, node-major bf16 rows
    hw1 = (x * osq[:, None]) @ W1
    tbl1 = np.zeros((p.TOT_ROWS, 128), _BF)
    rows = p.row_of(np.arange(N))
    tbl1[rows, 0:64] = hw1.astype(_BF)

    # per-node fold vectors, in stage layout [p, t] = node loc = t*128 + p
    sc12 = np.zeros((R, 128, p.TPR), np.float32)
    sc3 = np.zeros((R, 128, p.TPR), np.float32)
    for r in range(R):
        lo = r * p.NPR
        hi = min(lo + p.NPR, p.N)
        v12 = np.zeros(p.ROWS_PR, np.float32)
        v3 = np.zeros(p.ROWS_PR, np.float32)
        v12[:hi - lo] = (isq * osq)[lo:hi]
        v3[:hi - lo] = isq[lo:hi]
        sc12[r] = v12.reshape(p.TPR, 128).T
        sc3[r] = v3.reshape(p.TPR, 128).T

    # host-folded constants
    Ws = np.stack([W2, W3, Wfc1[:64], Wfc1[64:]], axis=1).astype(_BF)  # [64,4,64]
    wdiff = (Wfc2[:, 1] - Wfc2[:, 0]).astype(np.float32)
    bd = float(bfc2[1] - bfc2[0])
    wdbd = np.zeros((128, 65), np.float32)
    wdbd[:, 0:64] = wdiff[None, :]
    wdbd[:, 64] = bd
    bz = bfc1.astype(np.float32)          # pre-relu bias (z = u + v + bfc1)
    any_bz = bool(np.any(bz))
    iota_np = np.tile(np.arange(V, dtype=np.float32), (128, 1)).astype(_BF)

    nc = _build(p, any_bz)
    if not os.environ.get("GCN_SIM"):
        _split_excess_waits(nc)

    in_maps = []
    for r in range(R):
        m = {
            "tbl1": tbl1,
            "idxE": p.idx2[r], "dlE": p.dl2[r],
            "pidx1": p.pidx1[r], "pidx2": p.pidx2[r],
            "Ws": Ws, "wdbd": wdbd, "iotain": iota_np,
            "sc12": sc12[r],
            "sc3": sc3[r],
        }
        if any_bz:
            m["bz"] = np.tile(bz[None, :], (128, 1))
        in_maps.append(m)

    if os.environ.get("GCN_SIM"):
        from concourse.bass_interp import MultiCoreSim
        sim = MultiCoreSim(nc, R)
        for r in range(R):
            for k, v in in_maps[r].items():
                sim.cores[r].tensor(k)[:] = v
        sim.simulate()
        results = [{"pout": np.asarray(sim.cores[rr].mem_tensor("pout"))
                    .reshape(128, p.PCT, 2)} for rr in range(R)]

        class _R:
            pass
        res = _R()
        res.results = results
    else:
        res = run_bass_kernel_spmd(nc, in_maps, core_ids=list(range(R)),
                                   trace=_trace)

    out = np.zeros((p.NP, 2), np.float32)
    for r in range(R):
        po = np.asarray(res.results[r]["pout"]).reshape(128, p.PCT, 2)
        flat = po.transpose(1, 0, 2).reshape(-1, 2)   # slot j = c*128 + p
        valid = p.perm[r] >= 0
        out[p.perm[r][valid]] = flat[valid]
    if _trace:
        kernel.last_results = res
    return out


# revision 11
# speedup vs baseline: 1.1630x; 1.1630x over previous
"""3-layer GCN + gene-pair MLP on 8 Trainium2 NeuronCores (Bass/Tile).

Strategy (v2)
-------------
Nodes are sharded contiguously across the 8 cores by dst (12500 nodes each).
Edges live on the core that owns their dst node, bucketed by (dst-tile, band)
and padded only to the max count over cores.  Layer tables are node-major
bf16 rows padded to 256 B; layer-1's table (x*out_isqrt)@W1 is host-folded
(a linear transform of the inputs), so the kernel starts gathering
immediately.  Each layer:
  1. each core gathers table[src] rows for its edges with the SWDGE
     dma_gather, one gather per (tile, band) so the per-(t,b) padding can be
     trimmed via num_idxs_reg + trailing negative indices,
  2. aggregation is a one-hot matmul: S[e, v] = (dst_local[e] == v) built by
     a single DVE op (pad slots carry dl = -1 so their S row is zero), then
     aggT[f, v] += G[e, f]^T @ S accumulated in PSUM per 128-node tile,
  3. both GCN normalizations are folded into per-node scalings:
     out_isqrt premultiplies the staged table (host-side for layer 1,
     fused into the post-agg scale for layers 2/3), in_isqrt postmultiplies
     the aggregate; with zero biases relu(agg*isq)*osq == relu(agg*isq*osq)
     so one wide DVE multiply + relu per layer suffices.
  4. hw = h @ W is staged node-major and AllGathered into a shared full
     table for the next layer's gathers.  The first few gathers of the next
     layer are issued as SWDGE prepare_only on spare queues during the
     AllGather so the Q7 descriptor generator never idles; their triggers
     carry the deferred table dependency.
After layer 3 the kernel stages u = h3 @ Wfc1[:64], v = h3 @ Wfc1[64:] as one
packed [u|v] table; pairs gather u[gene1], v[gene2], and the 2-class softmax
collapses to sigmoid(z @ (Wfc2[:,1]-Wfc2[:,0]) + db).
"""
import sys
import os

sys.path.insert(0, "/opt/trn_rl_repo")

import numpy as np
import ml_dtypes

import concourse.bacc as bacc
import concourse.mybir as mybir
import concourse.tile as tile
from concourse.bass_utils import run_bass_kernel_spmd

bf16 = mybir.dt.bfloat16
f32 = mybir.dt.float32

R = int(os.environ.get("GCN_R", "8"))  # cores
V = 128          # nodes per aggregation tile
GT = int(os.environ.get("GCN_GT", "8"))   # tiles per gather group
MAXBAND = 30000  # int16-addressable rows per gather band (< 32768)
NPRE = int(os.environ.get("GCN_NPRE", "18"))   # prep-ahead gathers per layer
NPREP = int(os.environ.get("GCN_NPREP", "3"))  # prep-ahead pair buckets
GB = int(os.environ.get("GCN_GB", "6"))        # G/S pool depth

_BF = ml_dtypes.bfloat16


def _ceil(a, b):
    return -(-a // b)


def _wrap_idx(flat):
    """dma_gather index layout: position j -> [j % 16, j // 16], x8 partitions."""
    n = len(flat)
    assert n % 128 == 0
    arr = np.ascontiguousarray(flat.reshape(n // 16, 16).T.astype(np.int16))
    return np.tile(arr, (8, 1))


class _Plan:
    pass


def _make_plan(n_nodes, src, dst, gene1, gene2):
    p = _Plan()
    N = n_nodes
    NP = gene1.shape[0]
    p.N, p.NP = N, NP
    p.NPR = _ceil(N, R)               # nodes per rank
    p.TPR = _ceil(p.NPR, 128)         # node tiles per rank
    p.ROWS_PR = p.TPR * 128           # table rows per rank
    p.TOT_ROWS = p.ROWS_PR * R
    p.NB = max(1, _ceil(p.TOT_ROWS, MAXBAND))
    p.BSZ = _ceil(p.TOT_ROWS, p.NB)   # rows per band (last may be short)
    assert p.BSZ < 32768
    p.PPR = _ceil(NP, R)              # pairs per rank

    def row_of(n):
        r = n // p.NPR
        l = n - r * p.NPR
        return p.ROWS_PR * r + p.TPR * (l % 128) + (l // 128)

    p.row_of = row_of

    # ---- edge structure (shared across the 3 layers) ----
    own = (dst // p.NPR).astype(np.int64)
    loc = dst - own * p.NPR
    tl = loc // 128                      # tile within rank
    dl = (loc % 128).astype(np.float32)  # one-hot column
    rs = row_of(src)
    band = rs // p.BSZ
    ridx = (rs - band * p.BSZ).astype(np.int64)

    NBt = p.NB
    bid = (own * p.TPR + tl) * NBt + band
    counts = np.bincount(bid, minlength=R * p.TPR * NBt).reshape(R, p.TPR, NBt)
    p.cnts = counts
    p.Lmax = counts.max(axis=0)                    # [TPR, NB] true max count
    p.Pch = _ceil(p.Lmax, 128)                     # chunks per (tile, band)
    p.Pch = np.maximum(p.Pch, 1)

    # column/run offsets in (group, band, tile) order; GT tiles per gather
    p.NG = _ceil(p.TPR, GT)
    p.col_run = np.zeros((p.TPR, NBt), np.int64)
    p.gathers = []                                 # (g, b, c0, nch)
    col = 0
    for g in range(p.NG):
        ts = range(g * GT, min((g + 1) * GT, p.TPR))
        for b in range(NBt):
            c0 = col
            for t in ts:
                p.col_run[t, b] = col
                col += p.Pch[t, b]
            p.gathers.append((g, b, c0, col - c0))
    p.CT = int(col)
    E_pad = p.CT * 128

    # per-core flat slots
    order = np.argsort(bid, kind="stable")
    bid_s = bid[order]
    own_s = own[order]
    uniq, first = np.unique(bid_s, return_index=True)
    start_map = np.zeros(R * p.TPR * NBt, np.int64)
    start_map[uniq] = first
    i_within = np.arange(len(order)) - start_map[bid_s]
    tl_s, band_s = tl[order], band[order]
    slot = p.col_run[tl_s, band_s] * 128 + i_within

    p.idx2 = np.zeros((R, 128, p.CT * 8), np.int16)
    p.dl2 = np.zeros((R, 128, p.CT), _BF)
    ridx_s, dl_ss = ridx[order], dl[order]

    # pad slots fetch row 0 of their band (valid) with dl = -1 so their
    # one-hot row is zero and they contribute nothing.
    for r in range(R):
        m = own_s == r
        idx_flat = np.zeros(E_pad, np.int64)
        dl_flat = np.full(E_pad, -1.0, np.float32)
        idx_flat[slot[m]] = ridx_s[m]
        dl_flat[slot[m]] = dl_ss[m]
        p.dl2[r] = dl_flat.reshape(p.CT, 128).T.astype(_BF)
        blocks = []
        for (_, _, c0, nch) in p.gathers:
            blocks.append(_wrap_idx(idx_flat[c0 * 128:(c0 + nch) * 128]))
        p.idx2[r] = np.hstack(blocks)

    # ---- pair structure ----
    g1r, g2r = row_of(gene1), row_of(gene2)
    pb = (g1r // p.BSZ) * NBt + (g2r // p.BSZ)
    pown = np.arange(NP) // p.PPR
    NBK = NBt * NBt
    pcnt = np.bincount(pown * NBK + pb, minlength=R * NBK).reshape(R, NBK)
    p.Lmaxp = pcnt.max(axis=0)
    p.Pchp = np.maximum(_ceil(p.Lmaxp, 128), 1)    # chunks per bucket
    p.pcol = np.concatenate([[0], np.cumsum(p.Pchp)])
    p.PCT = int(p.pcol[-1])
    PP_pad = p.PCT * 128

    pbid = pown * NBK + pb
    porder = np.argsort(pbid, kind="stable")
    pbid_s = pbid[porder]
    pown_s = pown[porder]
    uq, fs = np.unique(pbid_s, return_index=True)
    smap = np.zeros(R * NBK, np.int64)
    smap[uq] = fs
    pi_within = np.arange(NP) - smap[pbid_s]
    pslot = p.pcol[pb[porder]] * 128 + pi_within

    pbase = np.zeros(PP_pad, np.int64)
    for bkt in range(NBK):
        c0, lm = int(p.pcol[bkt]), int(p.Lmaxp[bkt])
        pbase[c0 * 128 + lm: (c0 + p.Pchp[bkt]) * 128] = -1

    p.pidx1 = np.zeros((R, 128, p.PCT * 8), np.int16)
    p.pidx2 = np.zeros((R, 128, p.PCT * 8), np.int16)
    p.perm = np.full((R, PP_pad), -1, np.int64)
    r1 = (g1r - (g1r // p.BSZ) * p.BSZ)[porder]
    r2 = (g2r - (g2r // p.BSZ) * p.BSZ)[porder]
    for r in range(R):
        m = pown_s == r
        f1 = pbase.copy()
        f2 = pbase.copy()
        f1[pslot[m]] = r1[m]
        f2[pslot[m]] = r2[m]
        p.perm[r][pslot[m]] = porder[m]
        b1s, b2s = [], []
        for bkt in range(NBK):
            c0, nch = int(p.pcol[bkt]), int(p.Pchp[bkt])
            b1s.append(_wrap_idx(f1[c0 * 128:(c0 + nch) * 128]))
            b2s.append(_wrap_idx(f2[c0 * 128:(c0 + nch) * 128]))
        p.pidx1[r] = np.hstack(b1s)
        p.pidx2[r] = np.hstack(b2s)
    return p


def _build(p, any_bz):
    """Build the SPMD Bass program for plan `p`."""
    STOP = int(os.environ.get("GCN_STOP", "9"))
    PREP = bool(os.environ.get("GCN_PREP"))
    nc = bacc.Bacc("TRN2", num_devices=R, num_swdge_queues=4)
    NBt, NBK = p.NB, p.NB * p.NB

    tbl1_d = nc.dram_tensor("tbl1", [p.TOT_ROWS, 128], bf16, kind="ExternalInput")
    idx_d = nc.dram_tensor("idxE", [128, p.CT * 8], mybir.dt.int16, kind="ExternalInput")
    dl_d = nc.dram_tensor("dlE", [128, p.CT], bf16, kind="ExternalInput")
    pi1_d = nc.dram_tensor("pidx1", [128, p.PCT * 8], mybir.dt.int16, kind="ExternalInput")
    pi2_d = nc.dram_tensor("pidx2", [128, p.PCT * 8], mybir.dt.int16, kind="ExternalInput")
    Ws_d = nc.dram_tensor("Ws", [64, 4, 64], bf16, kind="ExternalInput")
    sc12_d = nc.dram_tensor("sc12", [128, p.TPR], f32, kind="ExternalInput")
    sc3_d = nc.dram_tensor("sc3", [128, p.TPR], f32, kind="ExternalInput")
    wdbd_d = nc.dram_tensor("wdbd", [128, 65], f32, kind="ExternalInput")
    iota_d = nc.dram_tensor("iotain", [128, V], bf16, kind="ExternalInput")
    bz_d = nc.dram_tensor("bz", [128, 64], f32, kind="ExternalInput") if any_bz else None
    pout_d = nc.dram_tensor("pout", [128, p.PCT, 2], f32, kind="ExternalOutput")

    rg = [list(range(R))]
    # (tile, band) gather order
    gorder = [(t, b) for t in range(p.TPR) for b in range(NBt)]

    with tile.TileContext(nc) as tc:
        with tc.tile_pool(name="dloc", bufs=1, space="DRAM") as dloc, \
             tc.tile_pool(name="sb", bufs=1) as sb, \
             tc.tile_pool(name="ps", bufs=1, space="PSUM") as ps:

            stage_dram = dloc.tile([128, p.TPR, 128], bf16)
            fulls = [dloc.tile([p.TOT_ROWS, 128], bf16, tag=f"full{i}",
                               name=f"full{i}", addr_space="Shared")
                     for i in range(3)]

            iota_t = sb.tile([128, V], bf16)
            idx_t = sb.tile([128, p.CT * 8], mybir.dt.int16)
            dl_t = sb.tile([128, p.CT], bf16)
            pi1_t = sb.tile([128, p.PCT * 8], mybir.dt.int16)
            pi2_t = sb.tile([128, p.PCT * 8], mybir.dt.int16)
            Ws_t = sb.tile([64, 4, 64], bf16)
            sc12_t = sb.tile([128, p.TPR], f32)
            sc3_t = sb.tile([128, p.TPR], f32)
            wdbd_t = sb.tile([128, 65], f32)
            for t_, d_ in ((iota_t, iota_d), (idx_t, idx_d), (dl_t, dl_d),
                           (pi1_t, pi1_d), (pi2_t, pi2_d),
                           (Ws_t, Ws_d), (sc12_t, sc12_d), (sc3_t, sc3_d),
                           (wdbd_t, wdbd_d)):
                nc.sync.dma_start(out=t_[:], in_=d_[:])
            bz_t = None
            if any_bz:
                bz_t = sb.tile([128, 64], f32)
                nc.sync.dma_start(out=bz_t[:], in_=bz_d[:])

            stage_sb = sb.tile([128, p.TPR, 128], bf16)
            nc.vector.memset(stage_sb[:], 0.0)

            maxPch = max(nch for (_, _, _, nch) in p.gathers)
            import itertools
            _gseq = itertools.count()

            def g_tile():
                return sb.tile([128, maxPch, 128], bf16, tag="G", bufs=GB,
                               name=f"G{next(_gseq)}")

            def s_tile(nch):
                return sb.tile([128, nch, V], bf16, tag="S", bufs=6,
                               name=f"S{next(_gseq)}")

            def emit_gather(table, gi, prepare=False, sem=None, queue=0):
                (_, b, c0, nch) = p.gathers[gi]
                Gt = g_tile()
                lo = b * p.BSZ
                hi = min(lo + p.BSZ, p.TOT_ROWS)
                nc.gpsimd.dma_gather(
                    out_ap=Gt[:, 0:nch, :], in_ap=table[lo:hi, :],
                    idxs_ap=idx_t[:, c0 * 8:(c0 + nch) * 8],
                    num_idxs=nch * 128, num_idxs_reg=nch * 128,
                    elem_size=128, single_packet=False,
                    prepare_only=prepare, sem=sem, queue_num=queue)
                return Gt

            def wmm_stage(src_tile, wi, half, sc_t):
                """hw[:, t] = sc * src_tile[:, t*128:...].T @ Ws[wi] into stage.

                sc is the per-node (per-partition here) norm fold: relu(s*x)
                = s*relu(x) and row scaling commutes with @W, so both GCN
                norms land here as one fused tensor_scalar multiply."""
                for t in range(p.TPR):
                    pm = ps.tile([128, 64], f32, tag="wm", space="PSUM", bufs=2)
                    nc.tensor.matmul(out=pm[:], lhsT=src_tile[:, t * 128:(t + 1) * 128],
                                     rhs=Ws_t[:, wi, :], start=True, stop=True)
                    nc.vector.tensor_scalar(
                        out=stage_sb[:, t, half * 64:half * 64 + 64], in0=pm[:],
                        scalar1=sc_t[:, t:t + 1], scalar2=None,
                        op0=mybir.AluOpType.mult)

            pre_G = {}

            def emit_preps(l, table):
                if not PREP or NPRE <= 0:
                    return {}
                per_q = _ceil(NPRE, 3)
                used = {}
                for i in range(min(NPRE, len(p.gathers))):
                    q = 1 + i // per_q
                    sem = nc.alloc_semaphore(f"prep_l{l}_{i}")
                    Gt = emit_gather(table, i, prepare=True, sem=sem, queue=q)
                    pre_G[(l, i)] = Gt
                    nch = p.gathers[i][3]
                    used.setdefault(q, []).append(Gt[:, 0:nch, :])
                return used

            def emit_layer(l, table):
                hT = sb.tile([64, p.ROWS_PR], bf16, tag="feat", bufs=2,
                             name=f"hT{next(_gseq)}")
                for g in range(p.NG):
                    ts = range(g * GT, min((g + 1) * GT, p.TPR))
                    Gs, Ss, c0s = {}, {}, {}
                    for gi, (gg, b, c0, nch) in enumerate(p.gathers):
                        if gg != g:
                            continue
                        c0s[b] = c0
                        if (l, gi) in pre_G:
                            Gt = pre_G.pop((l, gi))
                        else:
                            Gt = emit_gather(table, gi)
                        St = s_tile(nch)
                        dl_b = dl_t[:, c0:c0 + nch].unsqueeze(2).to_broadcast([128, nch, V])
                        iota_b = iota_t[:].unsqueeze(1).to_broadcast([128, nch, V])
                        nc.vector.tensor_tensor(out=St[:], in0=iota_b, in1=dl_b,
                                                op=mybir.AluOpType.is_equal)
                        Gs[b], Ss[b] = Gt, St
                    for t in ts:
                        acc = ps.tile([64, V], f32, tag="acc", space="PSUM", bufs=2)
                        nch_t = int(p.Pch[t, :].sum())
                        ki = 0
                        for b in range(NBt):
                            base = int(p.col_run[t, b] - c0s[b])
                            for k in range(int(p.Pch[t, b])):
                                nc.tensor.matmul(
                                    out=acc[:],
                                    lhsT=Gs[b][:, base + k, 0:64],
                                    rhs=Ss[b][:, base + k, :],
                                    start=(ki == 0), stop=(ki == nch_t - 1))
                                ki += 1
                        nc.vector.tensor_copy(hT[:, t * 128:(t + 1) * 128], acc[:])
                if l < 2:
                    nc.vector.tensor_scalar_max(hT[:], hT[:], 0.0)
                return hT

            pre_P = {}

            def emit_pair_gather(bkt, which, prepare=False, sem=None, queue=0):
                c0, nch = int(p.pcol[bkt]), int(p.Pchp[bkt])
                lm = int(p.Lmaxp[bkt])
                b = bkt // NBt if which == 0 else bkt % NBt
                pit = pi1_t if which == 0 else pi2_t
                tt = sb.tile([128, nch, 128], bf16, tag="UV", bufs=6,
                             name=f"UV{next(_gseq)}")
                lo = b * p.BSZ
                hi = min(lo + p.BSZ, p.TOT_ROWS)
                nc.gpsimd.dma_gather(
                    out_ap=tt[:], in_ap=fulls[2][lo:hi, :],
                    idxs_ap=pit[:, c0 * 8:(c0 + nch) * 8],
                    num_idxs=nch * 128, num_idxs_reg=lm,
                    elem_size=128, single_packet=False,
                    prepare_only=prepare, sem=sem, queue_num=queue)
                return tt

            # ---- layers ----
            table = tbl1_d
            for l in range(3):
                if STOP < l + 1:
                    break
                hT = emit_layer(l, table)
                if l < 2:
                    wmm_stage(hT, l, 0, sc12_t)
                    nxt = fulls[l]
                else:
                    wmm_stage(hT, 2, 0, sc3_t)   # u = h3 @ Wfc1[:64]
                    wmm_stage(hT, 3, 1, sc3_t)   # v = h3 @ Wfc1[64:]
                    nxt = fulls[2]
                nc.sync.dma_start(out=stage_dram[:], in_=stage_sb[:])
                nc.gpsimd.collective_compute(
                    "AllGather", mybir.AluOpType.bypass, replica_groups=rg,
                    ins=[stage_dram[:]], outs=[nxt[:]])
                # prep-ahead for the next consumer of `nxt`: emitted after the
                # AllGather so the deferred table-read dependency lands on the
                # trigger (the prep itself runs during the collective).
                if l < 2:
                    used_q = emit_preps(l + 1, nxt)
                else:
                    used_q = {}
                    if PREP and NPREP > 0:
                        for bi in range(min(NPREP, NBK)):
                            q = 1 + bi % 3
                            for which in (0, 1):
                                sem = nc.alloc_semaphore(f"prep_p{bi}_{which}")
                                tt = emit_pair_gather(
                                    bi, which, prepare=True, sem=sem, queue=q)
                                pre_P[(bi, which)] = tt
                                used_q.setdefault(q, []).append(tt[:])
                for q in sorted(used_q):
                    nc.gpsimd.trigger_dma(count=None, queue_num=q,
                                          signals_writable=used_q[q])
                table = nxt

            # ---- pair stage ----
            for bkt in range(NBK) if STOP >= 4 else []:
                nch = int(p.Pchp[bkt])
                if (bkt, 0) in pre_P:
                    Ut = pre_P.pop((bkt, 0))
                    Vt = pre_P.pop((bkt, 1))
                else:
                    Ut = emit_pair_gather(bkt, 0)
                    Vt = emit_pair_gather(bkt, 1)
                z = sb.tile([128, nch, 64], f32, tag="z", bufs=2)
                nc.vector.tensor_tensor(out=z[:], in0=Ut[:, :, 0:64],
                                        in1=Vt[:, :, 64:128],
                                        op=mybir.AluOpType.add)
                if any_bz:
                    nc.vector.tensor_tensor(
                        out=z[:], in0=z[:],
                        in1=bz_t[:].unsqueeze(1).to_broadcast([128, nch, 64]),
                        op=mybir.AluOpType.add)
                nc.vector.tensor_scalar_max(z[:], z[:], 0.0)
                nc.vector.tensor_tensor(
                    out=z[:], in0=z[:],
                    in1=wdbd_t[:, 0:64].unsqueeze(1).to_broadcast([128, nch, 64]),
                    op=mybir.AluOpType.mult)
                ds = sb.tile([128, nch], f32, tag="ds", bufs=2)
                nc.vector.tensor_reduce(out=ds[:], in_=z[:],
                                        axis=mybir.AxisListType.X,
                                        op=mybir.AluOpType.add)
                po = sb.tile([128, nch, 2], f32, tag="po", bufs=2)
                nc.scalar.activation(po[:, :, 1:2], ds[:].unsqueeze(2),
                                     mybir.ActivationFunctionType.Sigmoid,
                                     bias=wdbd_t[:, 64:65], scale=1.0)
                nc.vector.tensor_scalar(
                    out=po[:, :, 0:1], in0=po[:, :, 1:2],
                    scalar1=-1.0, scalar2=1.0,
                    op0=mybir.AluOpType.mult, op1=mybir.AluOpType.add)
                c0 = int(p.pcol[bkt])
                nc.sync.dma_start(out=pout_d[:, c0:c0 + nch, :], in_=po[:])
    nc.compile()
    return nc


def _split_excess_waits(nc, max_waits=1):
    """Walrus rejects >1 sem wait on queue instructions; hoist extras onto
    standalone EventSemaphore instructions placed just before."""
    for fn in nc.m.functions:
        for bb in fn.blocks:
            il = bb.instructions
            new_list = []
            changed = False
            for ins in il:
                si = ins.sync_info
                if si is not None and si.on_wait and len(si.on_wait) > max_waits:
                    waits = list(si.on_wait)
                    keep, excess = waits[:max_waits], waits[max_waits:]
                    for gi in range(0, len(excess), max_waits):
                        ev = mybir.InstEventSemaphore(
                            name=f"{ins.name}_wsplit{gi}", ins=[], outs=[])
                        ev.engine = ins.engine
                        ev.sync_info = mybir.SyncInfo(
                            on_wait=excess[gi:gi + max_waits], on_update=[])
                        new_list.append(ev)
                    ins.sync_info = mybir.SyncInfo(
                        on_wait=keep, on_update=list(si.on_update))
                    changed = True
                new_list.append(ins)
            if changed:
                bb.instructions = new_list


def kernel(x, src, dst, gene1, gene2, W1, b1, W2, b2, W3, b3,
           Wfc1, bfc1, Wfc2, bfc2, _trace=False):
    x = np.asarray(x, np.float32)
    src = np.asarray(src, np.int64)
    dst = np.asarray(dst, np.int64)
    gene1 = np.asarray(gene1, np.int64)
    gene2 = np.asarray(gene2, np.int64)
    W1, b1 = np.asarray(W1, np.float32), np.asarray(b1, np.float32)
    W2, b2 = np.asarray(W2, np.float32), np.asarray(b2, np.float32)
    W3, b3 = np.asarray(W3, np.float32), np.asarray(b3, np.float32)
    Wfc1, bfc1 = np.asarray(Wfc1, np.float32), np.asarray(bfc1, np.float32)
    Wfc2, bfc2 = np.asarray(Wfc2, np.float32), np.asarray(bfc2, np.float32)

    assert not (np.any(b1) or np.any(b2) or np.any(b3)), \
        "nonzero GCN biases not supported by the folded-norm fast path"

    N = x.shape[0]
    p = _make_plan(N, src, dst, gene1, gene2)

    # degree norms (host, structural)
    ones = np.ones(len(src), np.float32)
    out_deg = np.clip(np.bincount(src, weights=ones, minlength=N), 1.0, None)
    in_deg = np.clip(np.bincount(dst, weights=ones, minlength=N), 1.0, None)
    osq = (out_deg ** -0.5).astype(np.float32)
    isq = (in_deg ** -0.5).astype(np.float32)

    # layer-1 table host-folded: (x * osq) @ W1, node-major bf16 rows
    hw1 = (x * osq[:, None]) @ W1
    tbl1 = np.zeros((p.TOT_ROWS, 128), _BF)
    rows = p.row_of(np.arange(N))
    tbl1[rows, 0:64] = hw1.astype(_BF)

    # per-node fold vectors, in stage layout [p, t] = node loc = t*128 + p
    sc12 = np.zeros((R, 128, p.TPR), np.float32)
    sc3 = np.zeros((R, 128, p.TPR), np.float32)
    for r in range(R):
        lo = r * p.NPR
        hi = min(lo + p.NPR, p.N)
        v12 = np.zeros(p.ROWS_PR, np.float32)
        v3 = np.zeros(p.ROWS_PR, np.float32)
        v12[:hi - lo] = (isq * osq)[lo:hi]
        v3[:hi - lo] = isq[lo:hi]
        sc12[r] = v12.reshape(p.TPR, 128).T
        sc3[r] = v3.reshape(p.TPR, 128).T

    # host-folded constants
    Ws = np.stack([W2, W3, Wfc1[:64], Wfc1[64:]], axis=1).astype(_BF)  # [64,4,64]
    wdiff = (Wfc2[:, 1] - Wfc2[:, 0]).astype(np.float32)
    bd = float(bfc2[1] - bfc2[0])
    wdbd = np.zeros((128, 65), np.float32)
    wdbd[:, 0:64] = wdiff[None, :]
    wdbd[:, 64] = bd
    bz = bfc1.astype(np.float32)          # pre-relu bias (z = u + v + bfc1)
    any_bz = bool(np.any(bz))
    iota_np = np.tile(np.arange(V, dtype=np.float32), (128, 1)).astype(_BF)

    nc = _build(p, any_bz)
    if not os.environ.get("GCN_SIM"):
        _split_excess_waits(nc)

    in_maps = []
    for r in range(R):
        m = {
            "tbl1": tbl1,
            "idxE": p.idx2[r], "dlE": p.dl2[r],
            "pidx1": p.pidx1[r], "pidx2": p.pidx2[r],
            "Ws": Ws, "wdbd": wdbd, "iotain": iota_np,
            "sc12": sc12[r],
            "sc3": sc3[r],
        }
        if any_bz:
            m["bz"] = np.tile(bz[None, :], (128, 1))
        in_maps.append(m)

    if os.environ.get("GCN_SIM"):
        from concourse.bass_interp import MultiCoreSim
        sim = MultiCoreSim(nc, R)
        for r in range(R):
            for k, v in in_maps[r].items():
                sim.cores[r].tensor(k)[:] = v
        sim.simulate()
        results = [{"pout": np.asarray(sim.cores[rr].mem_tensor("pout"))
                    .reshape(128, p.PCT, 2)} for rr in range(R)]

        class _R:
            pass
        res = _R()
        res.results = results
    else:
        res = run_bass_kernel_spmd(nc, in_maps, core_ids=list(range(R)),
                                   trace=_trace)

    out = np.zeros((p.NP, 2), np.float32)
    for r in range(R):
        po = np.asarray(res.results[r]["pout"]).reshape(128, p.PCT, 2)
        flat = po.transpose(1, 0, 2).reshape(-1, 2)   # slot j = c*128 + p
        valid = p.perm[r] >= 0
        out[p.perm[r][valid]] = flat[valid]
    if _trace:
        kernel.last_results = res
    return out


# revision 14
# speedup vs baseline: 1.1906x; 1.0238x over previous
"""3-layer GCN + gene-pair MLP on 8 Trainium2 NeuronCores (Bass/Tile).

Strategy (v2)
-------------
Nodes are sharded contiguously across the 8 cores by dst (12500 nodes each).
Edges live on the core that owns their dst node, bucketed by (dst-tile, band)
and padded only to the max count over cores.  Layer tables are node-major
bf16 rows padded to 256 B; layer-1's table (x*out_isqrt)@W1 is host-folded
(a linear transform of the inputs), so the kernel starts gathering
immediately.  Each layer:
  1. each core gathers table[src] rows for its edges with the SWDGE
     dma_gather, one gather per (tile, band) so the per-(t,b) padding can be
     trimmed via num_idxs_reg + trailing negative indices,
  2. aggregation is a one-hot matmul: S[e, v] = (dst_local[e] == v) built by
     a single DVE op (pad slots carry dl = -1 so their S row is zero), then
     aggT[f, v] += G[e, f]^T @ S accumulated in PSUM per 128-node tile,
  3. both GCN normalizations are folded into per-node scalings:
     out_isqrt premultiplies the staged table (host-side for layer 1,
     fused into the post-agg scale for layers 2/3), in_isqrt postmultiplies
     the aggregate; with zero biases relu(agg*isq)*osq == relu(agg*isq*osq)
     so one wide DVE multiply + relu per layer suffices.
  4. hw = h @ W is staged node-major and AllGathered into a shared full
     table for the next layer's gathers.  The first few gathers of the next
     layer are issued as SWDGE prepare_only on spare queues during the
     AllGather so the Q7 descriptor generator never idles; their triggers
     carry the deferred table dependency.
After layer 3 the kernel stages u = h3 @ Wfc1[:64], v = h3 @ Wfc1[64:] as one
packed [u|v] table; pairs gather u[gene1], v[gene2], and the 2-class softmax
collapses to sigmoid(z @ (Wfc2[:,1]-Wfc2[:,0]) + db).
"""
import sys
import os

sys.path.insert(0, "/opt/trn_rl_repo")

import numpy as np
import ml_dtypes

import concourse.bacc as bacc
import concourse.mybir as mybir
import concourse.tile as tile
from concourse.bass_utils import run_bass_kernel_spmd

bf16 = mybir.dt.bfloat16
f32 = mybir.dt.float32

R = int(os.environ.get("GCN_R", "8"))  # cores
V = 128          # nodes per aggregation tile
GT = int(os.environ.get("GCN_GT", "8"))   # tiles per gather group
MAXBAND = 30000  # int16-addressable rows per gather band (< 32768)
NPRE = int(os.environ.get("GCN_NPRE", "6"))    # prep-ahead gathers per layer
NPREP = int(os.environ.get("GCN_NPREP", "3"))  # prep-ahead pair buckets
GB = int(os.environ.get("GCN_GB", "6"))        # G/S pool depth

_BF = ml_dtypes.bfloat16


def _ceil(a, b):
    return -(-a // b)


def _wrap_idx(flat):
    """dma_gather index layout: position j -> [j % 16, j // 16], x8 partitions."""
    n = len(flat)
    assert n % 128 == 0
    arr = np.ascontiguousarray(flat.reshape(n // 16, 16).T.astype(np.int16))
    return np.tile(arr, (8, 1))


class _Plan:
    pass


def _make_plan(n_nodes, src, dst, gene1, gene2):
    p = _Plan()
    N = n_nodes
    NP = gene1.shape[0]
    p.N, p.NP = N, NP
    p.NPR = _ceil(N, R)               # nodes per rank
    p.TPR = _ceil(p.NPR, 128)         # node tiles per rank
    p.ROWS_PR = p.TPR * 128           # table rows per rank
    p.TOT_ROWS = p.ROWS_PR * R
    p.NB = max(1, _ceil(p.TOT_ROWS, MAXBAND))
    p.BSZ = _ceil(p.TOT_ROWS, p.NB)   # rows per band (last may be short)
    assert p.BSZ < 32768
    p.PPR = _ceil(NP, R)              # pairs per rank

    def row_of(n):
        r = n // p.NPR
        l = n - r * p.NPR
        return p.ROWS_PR * r + p.TPR * (l % 128) + (l // 128)

    p.row_of = row_of

    # ---- edge structure (shared across the 3 layers) ----
    own = (dst // p.NPR).astype(np.int64)
    loc = dst - own * p.NPR
    tl = loc // 128                      # tile within rank
    dl = (loc % 128).astype(np.float32)  # one-hot column
    rs = row_of(src)
    band = rs // p.BSZ
    ridx = (rs - band * p.BSZ).astype(np.int64)

    NBt = p.NB
    bid = (own * p.TPR + tl) * NBt + band
    counts = np.bincount(bid, minlength=R * p.TPR * NBt).reshape(R, p.TPR, NBt)
    p.cnts = counts
    p.Lmax = counts.max(axis=0)                    # [TPR, NB] true max count
    p.Pch = _ceil(p.Lmax, 128)                     # chunks per (tile, band)
    p.Pch = np.maximum(p.Pch, 1)

    # column/run offsets in (group, band, tile) order; GT tiles per gather
    p.NG = _ceil(p.TPR, GT)
    p.col_run = np.zeros((p.TPR, NBt), np.int64)
    p.gathers = []                                 # (g, b, c0, nch)
    col = 0
    for g in range(p.NG):
        ts = range(g * GT, min((g + 1) * GT, p.TPR))
        for b in range(NBt):
            c0 = col
            for t in ts:
                p.col_run[t, b] = col
                col += p.Pch[t, b]
            p.gathers.append((g, b, c0, col - c0))
    p.CT = int(col)
    E_pad = p.CT * 128

    # per-core flat slots
    order = np.argsort(bid, kind="stable")
    bid_s = bid[order]
    own_s = own[order]
    uniq, first = np.unique(bid_s, return_index=True)
    start_map = np.zeros(R * p.TPR * NBt, np.int64)
    start_map[uniq] = first
    i_within = np.arange(len(order)) - start_map[bid_s]
    tl_s, band_s = tl[order], band[order]
    slot = p.col_run[tl_s, band_s] * 128 + i_within

    p.idx2 = np.zeros((R, 128, p.CT * 8), np.int16)
    p.dl2 = np.zeros((R, 128, p.CT), _BF)
    ridx_s, dl_ss = ridx[order], dl[order]

    # pad slots fetch row 0 of their band (valid) with dl = -1 so their
    # one-hot row is zero and they contribute nothing.
    for r in range(R):
        m = own_s == r
        idx_flat = np.zeros(E_pad, np.int64)
        dl_flat = np.full(E_pad, -1.0, np.float32)
        idx_flat[slot[m]] = ridx_s[m]
        dl_flat[slot[m]] = dl_ss[m]
        p.dl2[r] = dl_flat.reshape(p.CT, 128).T.astype(_BF)
        blocks = []
        for (_, _, c0, nch) in p.gathers:
            blocks.append(_wrap_idx(idx_flat[c0 * 128:(c0 + nch) * 128]))
        p.idx2[r] = np.hstack(blocks)

    # ---- pair structure ----
    g1r, g2r = row_of(gene1), row_of(gene2)
    pb = (g1r // p.BSZ) * NBt + (g2r // p.BSZ)
    pown = np.arange(NP) // p.PPR
    NBK = NBt * NBt
    pcnt = np.bincount(pown * NBK + pb, minlength=R * NBK).reshape(R, NBK)
    p.Lmaxp = pcnt.max(axis=0)
    p.Pchp = np.maximum(_ceil(p.Lmaxp, 128), 1)    # chunks per bucket
    p.pcol = np.concatenate([[0], np.cumsum(p.Pchp)])
    p.PCT = int(p.pcol[-1])
    PP_pad = p.PCT * 128

    pbid = pown * NBK + pb
    porder = np.argsort(pbid, kind="stable")
    pbid_s = pbid[porder]
    pown_s = pown[porder]
    uq, fs = np.unique(pbid_s, return_index=True)
    smap = np.zeros(R * NBK, np.int64)
    smap[uq] = fs
    pi_within = np.arange(NP) - smap[pbid_s]
    pslot = p.pcol[pb[porder]] * 128 + pi_within

    pbase = np.zeros(PP_pad, np.int64)
    for bkt in range(NBK):
        c0, lm = int(p.pcol[bkt]), int(p.Lmaxp[bkt])
        pbase[c0 * 128 + lm: (c0 + p.Pchp[bkt]) * 128] = -1

    p.pidx1 = np.zeros((R, 128, p.PCT * 8), np.int16)
    p.pidx2 = np.zeros((R, 128, p.PCT * 8), np.int16)
    p.perm = np.full((R, PP_pad), -1, np.int64)
    r1 = (g1r - (g1r // p.BSZ) * p.BSZ)[porder]
    r2 = (g2r - (g2r // p.BSZ) * p.BSZ)[porder]
    for r in range(R):
        m = pown_s == r
        f1 = pbase.copy()
        f2 = pbase.copy()
        f1[pslot[m]] = r1[m]
        f2[pslot[m]] = r2[m]
        p.perm[r][pslot[m]] = porder[m]
        b1s, b2s = [], []
        for bkt in range(NBK):
            c0, nch = int(p.pcol[bkt]), int(p.Pchp[bkt])
            b1s.append(_wrap_idx(f1[c0 * 128:(c0 + nch) * 128]))
            b2s.append(_wrap_idx(f2[c0 * 128:(c0 + nch) * 128]))
        p.pidx1[r] = np.hstack(b1s)
        p.pidx2[r] = np.hstack(b2s)
    return p


def _build(p, any_bz):
    """Build the SPMD Bass program for plan `p`."""
    STOP = int(os.environ.get("GCN_STOP", "9"))
    PREP = not os.environ.get("GCN_NOPREP")
    nc = bacc.Bacc("TRN2", num_devices=R, num_swdge_queues=4)
    NBt, NBK = p.NB, p.NB * p.NB

    tbl1_d = nc.dram_tensor("tbl1", [p.TOT_ROWS, 128], bf16, kind="ExternalInput")
    idx_d = nc.dram_tensor("idxE", [128, p.CT * 8], mybir.dt.int16, kind="ExternalInput")
    dl_d = nc.dram_tensor("dlE", [128, p.CT], bf16, kind="ExternalInput")
    pi1_d = nc.dram_tensor("pidx1", [128, p.PCT * 8], mybir.dt.int16, kind="ExternalInput")
    pi2_d = nc.dram_tensor("pidx2", [128, p.PCT * 8], mybir.dt.int16, kind="ExternalInput")
    Ws_d = nc.dram_tensor("Ws", [64, 4, 64], bf16, kind="ExternalInput")
    sc12_d = nc.dram_tensor("sc12", [128, p.TPR], f32, kind="ExternalInput")
    sc3_d = nc.dram_tensor("sc3", [128, p.TPR], f32, kind="ExternalInput")
    wdbd_d = nc.dram_tensor("wdbd", [128, 65], f32, kind="ExternalInput")
    iota_d = nc.dram_tensor("iotain", [128, V], bf16, kind="ExternalInput")
    bz_d = nc.dram_tensor("bz", [128, 64], f32, kind="ExternalInput") if any_bz else None
    pout_d = nc.dram_tensor("pout", [128, p.PCT, 2], f32, kind="ExternalOutput")

    rg = [list(range(R))]
    # (tile, band) gather order
    gorder = [(t, b) for t in range(p.TPR) for b in range(NBt)]

    with tile.TileContext(nc) as tc:
        with tc.tile_pool(name="dloc", bufs=1, space="DRAM") as dloc, \
             tc.tile_pool(name="sb", bufs=1) as sb, \
             tc.tile_pool(name="ps", bufs=1, space="PSUM") as ps:

            stage_dram = dloc.tile([128, p.TPR, 128], bf16)
            fulls = [dloc.tile([p.TOT_ROWS, 128], bf16, tag=f"full{i}",
                               name=f"full{i}", addr_space="Shared")
                     for i in range(3)]

            iota_t = sb.tile([128, V], bf16)
            idx_t = sb.tile([128, p.CT * 8], mybir.dt.int16)
            dl_t = sb.tile([128, p.CT], bf16)
            pi1_t = sb.tile([128, p.PCT * 8], mybir.dt.int16)
            pi2_t = sb.tile([128, p.PCT * 8], mybir.dt.int16)
            Ws_t = sb.tile([64, 4, 64], bf16)
            sc12_t = sb.tile([128, p.TPR], f32)
            sc3_t = sb.tile([128, p.TPR], f32)
            wdbd_t = sb.tile([128, 65], f32)
            for t_, d_ in ((iota_t, iota_d), (idx_t, idx_d), (dl_t, dl_d),
                           (pi1_t, pi1_d), (pi2_t, pi2_d),
                           (Ws_t, Ws_d), (sc12_t, sc12_d), (sc3_t, sc3_d),
                           (wdbd_t, wdbd_d)):
                nc.sync.dma_start(out=t_[:], in_=d_[:])
            bz_t = None
            if any_bz:
                bz_t = sb.tile([128, 64], f32)
                nc.sync.dma_start(out=bz_t[:], in_=bz_d[:])

            stage_sb = sb.tile([128, p.TPR, 128], bf16)
            nc.vector.memset(stage_sb[:], 0.0)

            maxPch = max(nch for (_, _, _, nch) in p.gathers)
            import itertools
            _gseq = itertools.count()

            def g_tile():
                return sb.tile([128, maxPch, 128], bf16, tag="G", bufs=GB,
                               name=f"G{next(_gseq)}")

            def s_tile(nch):
                return sb.tile([128, nch, V], bf16, tag="S", bufs=6,
                               name=f"S{next(_gseq)}")

            def emit_gather(table, gi, prepare=False, sem=None, queue=0):
                (_, b, c0, nch) = p.gathers[gi]
                Gt = g_tile()
                lo = b * p.BSZ
                hi = min(lo + p.BSZ, p.TOT_ROWS)
                nc.gpsimd.dma_gather(
                    out_ap=Gt[:, 0:nch, :], in_ap=table[lo:hi, :],
                    idxs_ap=idx_t[:, c0 * 8:(c0 + nch) * 8],
                    num_idxs=nch * 128, num_idxs_reg=nch * 128,
                    elem_size=128, single_packet=False,
                    prepare_only=prepare, sem=sem, queue_num=queue)
                return Gt

            def wmm_stage(src_tile, wi, half, sc_t):
                """hw[:, t] = sc * src_tile[:, t*128:...].T @ Ws[wi] into stage.

                sc is the per-node (per-partition here) norm fold: relu(s*x)
                = s*relu(x) and row scaling commutes with @W, so both GCN
                norms land here as one fused tensor_scalar multiply."""
                for t in range(p.TPR):
                    pm = ps.tile([128, 64], f32, tag="wm", space="PSUM", bufs=2)
                    nc.tensor.matmul(out=pm[:], lhsT=src_tile[:, t * 128:(t + 1) * 128],
                                     rhs=Ws_t[:, wi, :], start=True, stop=True)
                    nc.vector.tensor_scalar(
                        out=stage_sb[:, t, half * 64:half * 64 + 64], in0=pm[:],
                        scalar1=sc_t[:, t:t + 1], scalar2=None,
                        op0=mybir.AluOpType.mult)

            pre_G = {}

            def emit_preps(l, table):
                if not PREP or NPRE <= 0:
                    return {}
                per_q = _ceil(NPRE, 3)
                used = {}
                for i in range(min(NPRE, len(p.gathers))):
                    q = 1 + i // per_q
                    sem = nc.alloc_semaphore(f"prep_l{l}_{i}")
                    nc.gpsimd.sem_clear(sem)
                    Gt = emit_gather(table, i, prepare=True, sem=sem, queue=q)
                    pre_G[(l, i)] = (Gt, sem)
                    used.setdefault(q, []).append(q)
                return used

            def emit_layer(l, table):
                hT = sb.tile([64, p.ROWS_PR], bf16, tag="feat", bufs=2,
                             name=f"hT{next(_gseq)}")
                for g in range(p.NG):
                    ts = range(g * GT, min((g + 1) * GT, p.TPR))
                    Gs, Ss, c0s = {}, {}, {}
                    for gi, (gg, b, c0, nch) in enumerate(p.gathers):
                        if gg != g:
                            continue
                        c0s[b] = c0
                        if (l, gi) in pre_G:
                            Gt, psem = pre_G.pop((l, gi))
                            # the prep's DMA fires at trigger time; Tile only
                            # orders consumers against the prep itself, so
                            # wait for the descriptor-baked DMA sem here.
                            nc.tensor.wait_ge(psem, 16)
                        else:
                            Gt = emit_gather(table, gi)
                        St = s_tile(nch)
                        dl_b = dl_t[:, c0:c0 + nch].unsqueeze(2).to_broadcast([128, nch, V])
                        iota_b = iota_t[:].unsqueeze(1).to_broadcast([128, nch, V])
                        nc.vector.tensor_tensor(out=St[:], in0=iota_b, in1=dl_b,
                                                op=mybir.AluOpType.is_equal)
                        Gs[b], Ss[b] = Gt, St
                    for t in ts:
                        acc = ps.tile([64, V], f32, tag="acc", space="PSUM", bufs=2)
                        nch_t = int(p.Pch[t, :].sum())
                        ki = 0
                        for b in range(NBt):
                            base = int(p.col_run[t, b] - c0s[b])
                            for k in range(int(p.Pch[t, b])):
                                nc.tensor.matmul(
                                    out=acc[:],
                                    lhsT=Gs[b][:, base + k, 0:64],
                                    rhs=Ss[b][:, base + k, :],
                                    start=(ki == 0), stop=(ki == nch_t - 1))
                                ki += 1
                        nc.vector.tensor_copy(hT[:, t * 128:(t + 1) * 128], acc[:])
                if l < 2:
                    nc.vector.tensor_scalar_max(hT[:], hT[:], 0.0)
                return hT

            pre_P = {}

            def emit_pair_gather(bkt, which, prepare=False, sem=None, queue=0):
                c0, nch = int(p.pcol[bkt]), int(p.Pchp[bkt])
                lm = int(p.Lmaxp[bkt])
                b = bkt // NBt if which == 0 else bkt % NBt
                pit = pi1_t if which == 0 else pi2_t
                tt = sb.tile([128, nch, 128], bf16, tag="UV", bufs=6,
                             name=f"UV{next(_gseq)}")
                lo = b * p.BSZ
                hi = min(lo + p.BSZ, p.TOT_ROWS)
                nc.gpsimd.dma_gather(
                    out_ap=tt[:], in_ap=fulls[2][lo:hi, :],
                    idxs_ap=pit[:, c0 * 8:(c0 + nch) * 8],
                    num_idxs=nch * 128, num_idxs_reg=lm,
                    elem_size=128, single_packet=False,
                    prepare_only=prepare, sem=sem, queue_num=queue)
                return tt

            # ---- layers ----
            table = tbl1_d
            for l in range(3):
                if STOP < l + 1:
                    break
                hT = emit_layer(l, table)
                if l < 2:
                    wmm_stage(hT, l, 0, sc12_t)
                    nxt = fulls[l]
                else:
                    wmm_stage(hT, 2, 0, sc3_t)   # u = h3 @ Wfc1[:64]
                    wmm_stage(hT, 3, 1, sc3_t)   # v = h3 @ Wfc1[64:]
                    nxt = fulls[2]
                nc.sync.dma_start(out=stage_dram[:], in_=stage_sb[:])
                nc.gpsimd.collective_compute(
                    "AllGather", mybir.AluOpType.bypass, replica_groups=rg,
                    ins=[stage_dram[:]], outs=[nxt[:]])
                # prep-ahead for the next consumer of `nxt`: emitted after the
                # AllGather so the deferred table-read dependency lands on the
                # trigger (the prep itself runs during the collective).
                if l < 2:
                    used_q = emit_preps(l + 1, nxt)
                else:
                    used_q = {}
                    if PREP and NPREP > 0:
                        for bi in range(min(NPREP, NBK)):
                            q = 1 + bi % 3
                            for which in (0, 1):
                                sem = nc.alloc_semaphore(f"prep_p{bi}_{which}")
                                nc.gpsimd.sem_clear(sem)
                                tt = emit_pair_gather(
                                    bi, which, prepare=True, sem=sem, queue=q)
                                pre_P[(bi, which)] = (tt, sem)
                                used_q.setdefault(q, []).append(q)
                for q in sorted(used_q):
                    nc.gpsimd.trigger_dma(count=None, queue_num=q)
                table = nxt

            # ---- pair stage ----
            for bkt in range(NBK) if STOP >= 4 else []:
                nch = int(p.Pchp[bkt])
                if (bkt, 0) in pre_P:
                    Ut, s1 = pre_P.pop((bkt, 0))
                    Vt, s2 = pre_P.pop((bkt, 1))
                    nc.vector.wait_ge(s1, 16)
                    nc.vector.wait_ge(s2, 16)
                else:
                    Ut = emit_pair_gather(bkt, 0)
                    Vt = emit_pair_gather(bkt, 1)
                z = sb.tile([128, nch, 64], f32, tag="z", bufs=2)
                nc.vector.tensor_tensor(out=z[:], in0=Ut[:, :, 0:64],
                                        in1=Vt[:, :, 64:128],
                                        op=mybir.AluOpType.add)
                if any_bz:
                    nc.vector.tensor_tensor(
                        out=z[:], in0=z[:],
                        in1=bz_t[:].unsqueeze(1).to_broadcast([128, nch, 64]),
                        op=mybir.AluOpType.add)
                nc.vector.tensor_scalar_max(z[:], z[:], 0.0)
                nc.vector.tensor_tensor(
                    out=z[:], in0=z[:],
                    in1=wdbd_t[:, 0:64].unsqueeze(1).to_broadcast([128, nch, 64]),
                    op=mybir.AluOpType.mult)
                ds = sb.tile([128, nch], f32, tag="ds", bufs=2)
                nc.vector.tensor_reduce(out=ds[:], in_=z[:],
                                        axis=mybir.AxisListType.X,
                                        op=mybir.AluOpType.add)
                po = sb.tile([128, nch, 2], f32, tag="po", bufs=2)
                nc.scalar.activation(po[:, :, 1:2], ds[:].unsqueeze(2),
                                     mybir.ActivationFunctionType.Sigmoid,
                                     bias=wdbd_t[:, 64:65], scale=1.0)
                nc.vector.tensor_scalar(
                    out=po[:, :, 0:1], in0=po[:, :, 1:2],
                    scalar1=-1.0, scalar2=1.0,
                    op0=mybir.AluOpType.mult, op1=mybir.AluOpType.add)
                c0 = int(p.pcol[bkt])
                nc.sync.dma_start(out=pout_d[:, c0:c0 + nch, :], in_=po[:])
    nc.compile()
    return nc


def _split_excess_waits(nc, max_waits=1):
    """Walrus rejects >1 sem wait on queue instructions; hoist extras onto
    standalone EventSemaphore instructions placed just before."""
    for fn in nc.m.functions:
        for bb in fn.blocks:
            il = bb.instructions
            new_list = []
            changed = False
            for ins in il:
                si = ins.sync_info
                if si is not None and si.on_wait and len(si.on_wait) > max_waits:
                    waits = list(si.on_wait)
                    keep, excess = waits[:max_waits], waits[max_waits:]
                    for gi in range(0, len(excess), max_waits):
                        ev = mybir.InstEventSemaphore(
                            name=f"{ins.name}_wsplit{gi}", ins=[], outs=[])
                        ev.engine = ins.engine
                        ev.sync_info = mybir.SyncInfo(
                            on_wait=excess[gi:gi + max_waits], on_update=[])
                        new_list.append(ev)
                    ins.sync_info = mybir.SyncInfo(
                        on_wait=keep, on_update=list(si.on_update))
                    changed = True
                new_list.append(ins)
            if changed:
                bb.instructions = new_list


def kernel(x, src, dst, gene1, gene2, W1, b1, W2, b2, W3, b3,
           Wfc1, bfc1, Wfc2, bfc2, _trace=False):
    x = np.asarray(x, np.float32)
    src = np.asarray(src, np.int64)
    dst = np.asarray(dst, np.int64)
    gene1 = np.asarray(gene1, np.int64)
    gene2 = np.asarray(gene2, np.int64)
    W1, b1 = np.asarray(W1, np.float32), np.asarray(b1, np.float32)
    W2, b2 = np.asarray(W2, np.float32), np.asarray(b2, np.float32)
    W3, b3 = np.asarray(W3, np.float32), np.asarray(b3, np.float32)
    Wfc1, bfc1 = np.asarray(Wfc1, np.float32), np.asarray(bfc1, np.float32)
    Wfc2, bfc2 = np.asarray(Wfc2, np.float32), np.asarray(bfc2, np.float32)

    assert not (np.any(b1) or np.any(b2) or np.any(b3)), \
        "nonzero GCN biases not supported by the folded-norm fast path"

    N = x.shape[0]
    p = _make_plan(N, src, dst, gene1, gene2)

    # degree norms (host, structural)
    ones = np.ones(len(src), np.float32)
    out_deg = np.clip(np.bincount(src, weights=ones, minlength=N), 1.0, None)
    in_deg = np.clip(np.bincount(dst, weights=ones, minlength=N), 1.0, None)
    osq = (out_deg ** -0.5).astype(np.float32)
    isq = (in_deg ** -0.5).astype(np.float32)

    # layer-1 table host-folded: (x * osq) @ W1, node-major bf16 rows
    hw1 = (x * osq[:, None]) @ W1
    tbl1 = np.zeros((p.TOT_ROWS, 128), _BF)
    rows = p.row_of(np.arange(N))
    tbl1[rows, 0:64] = hw1.astype(_BF)

    # per-node fold vectors, in stage layout [p, t] = node loc = t*128 + p
    sc12 = np.zeros((R, 128, p.TPR), np.float32)
    sc3 = np.zeros((R, 128, p.TPR), np.float32)
    for r in range(R):
        lo = r * p.NPR
        hi = min(lo + p.NPR, p.N)
        v12 = np.zeros(p.ROWS_PR, np.float32)
        v3 = np.zeros(p.ROWS_PR, np.float32)
        v12[:hi - lo] = (isq * osq)[lo:hi]
        v3[:hi - lo] = isq[lo:hi]
        sc12[r] = v12.reshape(p.TPR, 128).T
        sc3[r] = v3.reshape(p.TPR, 128).T

    # host-folded constants
    Ws = np.stack([W2, W3, Wfc1[:64], Wfc1[64:]], axis=1).astype(_BF)  # [64,4,64]
    wdiff = (Wfc2[:, 1] - Wfc2[:, 0]).astype(np.float32)
    bd = float(bfc2[1] - bfc2[0])
    wdbd = np.zeros((128, 65), np.float32)
    wdbd[:, 0:64] = wdiff[None, :]
    wdbd[:, 64] = bd
    bz = bfc1.astype(np.float32)          # pre-relu bias (z = u + v + bfc1)
    any_bz = bool(np.any(bz))
    iota_np = np.tile(np.arange(V, dtype=np.float32), (128, 1)).astype(_BF)

    nc = _build(p, any_bz)
    if not os.environ.get("GCN_SIM"):
        _split_excess_waits(nc)

    in_maps = []
    for r in range(R):
        m = {
            "tbl1": tbl1,
            "idxE": p.idx2[r], "dlE": p.dl2[r],
            "pidx1": p.pidx1[r], "pidx2": p.pidx2[r],
            "Ws": Ws, "wdbd": wdbd, "iotain": iota_np,
            "sc12": sc12[r],
            "sc3": sc3[r],
        }
        if any_bz:
            m["bz"] = np.tile(bz[None, :], (128, 1))
        in_maps.append(m)

    if os.environ.get("GCN_SIM"):
        from concourse.bass_interp import MultiCoreSim
        sim = MultiCoreSim(nc, R)
        for r in range(R):
            for k, v in in_maps[r].items():
                sim.cores[r].tensor(k)[:] = v
        sim.simulate()
        results = [{"pout": np.asarray(sim.cores[rr].mem_tensor("pout"))
                    .reshape(128, p.PCT, 2)} for rr in range(R)]

        class _R:
            pass
        res = _R()
        res.results = results
    else:
        res = run_bass_kernel_spmd(nc, in_maps, core_ids=list(range(R)),
                                   trace=_trace)

    out = np.zeros((p.NP, 2), np.float32)
    for r in range(R):
        po = np.asarray(res.results[r]["pout"]).reshape(128, p.PCT, 2)
        flat = po.transpose(1, 0, 2).reshape(-1, 2)   # slot j = c*128 + p
        valid = p.perm[r] >= 0
        out[p.perm[r][valid]] = flat[valid]
    if _trace:
        kernel.last_results = res
    return out
